# revision 57
# baseline (speedup 1.0000x reference)
"""Fastmax (p=1 causal linear attention) Trainium2 kernel, 8-core SPMD, v2.

Sharding: data-parallel over heads (16 heads -> 2 per core). Each core
computes q/k/v projections for its 2 heads, chunked causal linear attention
(augmented [65,65] prefix state per head), and a partial output projection;
host sums the 8 partial outputs + bias.

v2 structure (vs v1): barrier-free scheduling. q/k stay UNSCALED through the
projections and score matmuls; the per-head normalizer s = 1/(qn*kn) is
applied only in a late fused op  masked = (scoresT * s) * maskf , and the
a0 (=1) additive term of fastmax scores is produced by constant-mask matmuls
(tri/ones) accumulated into the same PSUM tile as the score term. The prefix
state snapshot is scaled by s on its k-dim rows at snapshot-copy time. This
removes the global norm barrier from everything except the cheap tail ops.

Layouts (per core):
  qh[h]      [65, 2048]   transposed aug q (row 64 = ones) per head
  kh2        [128, 2048]  transposed k, both heads stacked (no ones row)
  krows/vrows[128, 16*136] token-major rows; per chunk [h0 64|1|h1 64|1|pad]
  vht        [128, 2048]  attention output (v-dims x tokens), heads stacked
PE work: projections q,k (direct), v (phase B token-major), krows via PE
transpose of kh2; scores kh2^T qh; per-chunk o accumulates mask-const
matmuls (a0 part), masked-score matmuls (s part) and the state application.
"""

import sys

sys.path.insert(0, "/opt/trn_rl_repo")

import numpy as np

B, N, D_MODEL, H, D_HEAD = 1, 2048, 1024, 16, 64
NCORES = 8
HPC = H // NCORES  # heads per core
DPC = HPC * D_HEAD  # out dims per core (128)
CH = 128  # chunk (tokens)
SPAN = 256  # query span (2 chunks)
NSPAN = N // SPAN
NCH = N // CH
KT = D_MODEL // 128  # contraction tiles for projections
NT = N // 512  # 512-wide column chunks of the sequence
RST = 136  # row-buffer stride per chunk (2*(64 data + ones col) + pad)

_CACHE = {}


def _build():
    import concourse.bass as bass
    import concourse.tile as tile
    import concourse.mybir as mybir
    from concourse import bacc
    from concourse.alu_op_type import AluOpType
    from concourse import bass_isa

    BF = mybir.dt.bfloat16
    F32 = mybir.dt.float32
    AF = mybir.ActivationFunctionType
    AX = mybir.AxisListType

    nc = bacc.Bacc("TRN2", target_bir_lowering=False, debug=False, num_devices=NCORES)

    xt_d = nc.declare_dram_parameter("xt", [D_MODEL, N], BF, isOutput=False)
    wq_d = nc.declare_dram_parameter("wq", [128, D_MODEL], BF, isOutput=False)
    wk_d = nc.declare_dram_parameter("wk", [128, D_MODEL], BF, isOutput=False)
    wv_d = nc.declare_dram_parameter("wv", [128, D_MODEL], BF, isOutput=False)
    wo_d = nc.declare_dram_parameter("wo", [DPC, D_MODEL], BF, isOutput=False)
    consts_d = nc.declare_dram_parameter("consts", [128, 644], BF, isOutput=False)
    out_d = nc.declare_dram_parameter("out", [N, D_MODEL], BF, isOutput=True)

    # engine load balancer for PSUM->SBUF / SBUF->SBUF copies & small ops
    load = {"v": 0.0, "s": 0.0, "g": 0.0}

    def _cost(eng, n, psum_src):
        if eng == "v":
            return n * (1.04 if psum_src else 0.3) + 140
        if eng == "s":
            return n * 0.84 + 195
        return n * 1.39 + 100

    with tile.TileContext(nc) as tc:

        def pick(n, psum_src=True, cand=("v", "s")):
            eng = min(cand, key=lambda e: load[e] + _cost(e, n, psum_src))
            load[eng] += _cost(eng, n, psum_src)
            return eng

        def acct(eng, n, psum_src=True):
            load[eng] += _cost(eng, n, psum_src)

        def rot(dst, src, n, psum_src=True):
            # gpsimd cannot run tensor ops through this walrus build
            eng = pick(n, psum_src)
            if eng == "v":
                nc.vector.tensor_copy(dst, src)
            else:
                nc.scalar.copy(dst, src)

        with (
            tc.tile_pool(name="const", bufs=1) as constp,
            tc.tile_pool(name="wqkv", bufs=1) as wp,
            tc.tile_pool(name="acts", bufs=1) as actp,
            tc.tile_pool(name="sct", bufs=1) as sctp,
            tc.tile_pool(name="chain", bufs=1) as chainp,
        ):
            consts = constp.tile([128, 644], BF)
            ident = consts[:, 0:128]
            onesall = consts[:, 128:256]
            maskf = consts[:, 256:640]  # [tri 128 | ones 128 | tri 128]
            tri = consts[:, 256:384]
            ones128 = consts[:, 384:512]
            hindt = consts[:, 640:642]

            # warm up Act function tables (Square/Sqrt/Copy) off the critical path
            warm = actp.tile([1, 1], F32, tag="warm")
            nc.gpsimd.memset(warm[:], 1.0)
            warm2 = actp.tile([1, 1], F32, tag="warm2")
            nc.scalar.activation(warm2[:], warm[:], AF.Square)
            nc.scalar.activation(warm2[:], warm[:], AF.Sqrt)

            # persistent activations (q and k transposed, heads stacked, no ones rows)
            qh2 = actp.tile([128, N], BF, tag="qh2")
            kh2 = actp.tile([128, N], BF, tag="kh2")
            vht = actp.tile([128, N], BF, tag="vht")
            krows = actp.tile([128, NCH * RST], BF, tag="krows")
            vrows = actp.tile([128, NCH * RST], BF, tag="vrows")
            # ones columns (64 and 129 of each chunk block) via full-tile fill
            nc.gpsimd.memset(krows[:], 1.0)
            nc.gpsimd.memset(vrows[:], 1.0)
            nrmbuf = actp.tile([2, 2 * NT], F32, tag="nrmbuf")
            scv = [actp.tile([128, 1], F32, tag=f"scv{h}", name=f"scv{h}") for h in range(HPC)]
            s_chain = {}
            chains = {}
            mfs = {}

            # weights and X tiles, issued in first-use order
            wq_sb = wp.tile([128, D_MODEL], BF)
            nc.sync.dma_start(wq_sb[:], wq_d[:])
            wk_sb = wp.tile([128, D_MODEL], BF)
            wv_sb = wp.tile([128, D_MODEL], BF)
            xref = {}
            for n0 in (0, 1):
                for k in range(KT):
                    xtile = actp.tile([128, 512], BF, tag=f"xt{k}_{n0}", name=f"xt{k}_{n0}")
                    nc.sync.dma_start(
                        xtile[:], xt_d[k * 128 : (k + 1) * 128, n0 * 512 : (n0 + 1) * 512]
                    )
                    xref[(k, n0)] = (xtile, 0)
                    if n0 == 0 and k == 0:
                        nc.sync.dma_start(wk_sb[:], wk_d[:])
                    if n0 == 0 and k == 1:
                        nc.sync.dma_start(consts[:], consts_d[:])
                    if n0 == 0 and k == 7:
                        nc.sync.dma_start(wv_sb[:], wv_d[:])
            for k in range(KT):
                xtile = actp.tile([128, 1024], BF, tag=f"xt{k}_23", name=f"xt{k}_23")
                nc.sync.dma_start(xtile[:], xt_d[k * 128 : (k + 1) * 128, 1024:2048])
                xref[(k, 2)] = (xtile, 0)
                xref[(k, 3)] = (xtile, 512)
            wo_sb = wp.tile([128, D_MODEL], BF)
            nc.sync.dma_start(wo_sb[:], wo_d[:])

            def xap(k, n0, c0=0, w=512):
                t, off = xref[(k, n0)]
                return t[:, off + c0 : off + c0 + w]

            # ====== pass 1: projections, rows, transposes, norms, sweeps ======
            with (
                tc.tile_pool(name="pp", bufs=2, space="PSUM") as pp,
                tc.tile_pool(name="rpp", bufs=2, space="PSUM") as rpp,
                tc.tile_pool(name="ktpp", bufs=1, space="PSUM") as ktpp,
                tc.tile_pool(name="dlp", bufs=1, space="PSUM") as dlp,
                tc.tile_pool(name="nrmp", bufs=1, space="PSUM") as nrmp,
                tc.tile_pool(name="sqp", bufs=2) as sqp,
            ):
                for n0 in range(NT):
                    cs = slice(n0 * 512, (n0 + 1) * 512)
                    for j, (wsb, nm) in enumerate(((wq_sb, "pq"), (wk_sb, "pk"))):
                        p = pp.tile([128, 512], F32, tag="p", name=nm)
                        for k in range(KT):
                            nc.tensor.matmul(
                                p[:],
                                wsb[:, k * 128 : (k + 1) * 128],
                                xap(k, n0),
                                start=(k == 0),
                                stop=(k == KT - 1),
                            )
                        rot((qh2 if j == 0 else kh2)[:, cs], p[:], 512)
                        sq = sqp.tile([128, 512], BF, tag="sq", name="sq")
                        nc.scalar.activation(sq[:], p[:], AF.Square)
                        acct("s", 512)
                        nrm = nrmp.tile([2, 512], F32, tag="nrm", name="nrm")
                        nc.tensor.matmul(nrm[:], hindt, sq[:], start=True, stop=True)
                        nc.vector.tensor_reduce(
                            nrmbuf[:, j * NT + n0 : j * NT + n0 + 1],
                            nrm[:],
                            AX.X,
                            AluOpType.max,
                        )
                        acct("v", 512)

                    if n0 == NT - 1:
                        # finalize rs = 1/sqrt(max|q|^2 * max|k|^2) early; the
                        # per-head broadcast (needs PSUM) happens after pass 1
                        nf = actp
                        mq = nf.tile([2, 1], F32, tag="mq")
                        mk = nf.tile([2, 1], F32, tag="mk")
                        nc.vector.tensor_reduce(mq[:], nrmbuf[:, 0:NT], AX.X, AluOpType.max)
                        nc.vector.tensor_reduce(
                            mk[:], nrmbuf[:, NT : 2 * NT], AX.X, AluOpType.max
                        )
                        prod = nf.tile([2, 1], F32, tag="prod")
                        nc.vector.tensor_mul(prod[:], mq[:], mk[:])
                        rt = nf.tile([2, 1], F32, tag="rt")
                        nc.scalar.activation(rt[:], prod[:], AF.Sqrt)
                        rs = nf.tile([2, 1], F32, tag="rs")
                        nc.vector.reciprocal(rs[:], rt[:])
                        rsb = nf.tile([2, 1], BF, tag="rsb")
                        nc.vector.tensor_copy(rsb[:], rs[:])

                    # v rows, k/q transposes, per-token norm^2 accumulation
                    for c in range(4 * n0, 4 * n0 + 4):
                        ts = c % 4
                        ck = slice(c * CH, (c + 1) * CH)
                        rp = rpp.tile([128, 128], F32, tag="rp", name="rp")
                        for k in range(KT):
                            nc.tensor.matmul(
                                rp[:],
                                xap(k, n0, ts * 128, 128),
                                wv_sb[:, k * 128 : (k + 1) * 128],
                                start=(k == 0),
                                stop=(k == KT - 1),
                            )
                        vdst = bass.AP(
                            vrows[:].tensor,
                            vrows[:].offset + c * RST,
                            [[NCH * RST, 128], [65, 2], [1, 64]],
                        )
                        vsrc = bass.AP(
                            rp[:].tensor, rp[:].offset, [[128, 128], [64, 2], [1, 64]]
                        )
                        rot(vdst, vsrc, 128)
                        ktp = ktpp.tile([128, 128], BF, tag="ktp", name="ktp")
                        nc.tensor.transpose(ktp[:], kh2[:, ck], ident)
                        kdst = bass.AP(
                            krows[:].tensor,
                            krows[:].offset + c * RST,
                            [[NCH * RST, 128], [65, 2], [1, 64]],
                        )
                        ksrc = bass.AP(
                            ktp[:].tensor, ktp[:].offset, [[128, 128], [64, 2], [1, 64]]
                        )
                        rot(kdst, ksrc, 128)

                    # state sweep: accumulate prefix state in PSUM, snapshot bf16
                    for sp in (2 * n0, 2 * n0 + 1):
                        if sp < NSPAN - 1:
                            for h in range(HPC):
                                ca, cb = 2 * sp, 2 * sp + 1
                                if sp == 0:
                                    chains[h] = dlp.tile(
                                        [65, 65], F32, tag=f"chain{h}", name=f"chain{h}"
                                    )
                                chn = chains[h]
                                nc.tensor.matmul(
                                    chn[:],
                                    krows[:, ca * RST + h * 65 : ca * RST + h * 65 + 65],
                                    vrows[:, ca * RST + h * 65 : ca * RST + h * 65 + 65],
                                    start=(sp == 0),
                                    stop=False,
                                    skip_group_check=True,
                                )
                                nc.tensor.matmul(
                                    chn[:],
                                    krows[:, cb * RST + h * 65 : cb * RST + h * 65 + 65],
                                    vrows[:, cb * RST + h * 65 : cb * RST + h * 65 + 65],
                                    start=False,
                                    stop=True,
                                    skip_group_check=True,
                                )
                                su = chainp.tile(
                                    [65, 65], BF, tag=f"su{h}_{sp}", name=f"su{h}_{sp}"
                                )
                                rot(su[:], chn[:], 65)
                                s_chain[(h, sp)] = su

            # ====== finalize: broadcast per-head scale to all partitions ======
            with tc.tile_pool(name="finp", bufs=2, space="PSUM") as finp:
                nf = actp
                rst_p = finp.tile([1, 2], BF, tag="t", name="rst_p")
                nc.tensor.transpose(rst_p[:], rsb[:], ident[0:2, 0:2])
                rs12 = nf.tile([1, 2], BF, tag="rs12")
                nc.vector.tensor_copy(rs12[:], rst_p[:])
                for h in range(HPC):
                    sb = finp.tile([128, 1], F32, tag="b", name="sb")
                    nc.tensor.matmul(
                        sb[:], onesall[0:1, 0:128], rs12[0:1, h : h + 1],
                        start=True, stop=True,
                    )
                    nc.vector.tensor_copy(scv[h][:], sb[:])

            # ====== phase 3: masked attention + fused outproj (s-gated) ======
            # Software pipeline: attention(sp) overlaps vht+outproj(sp-1);
            # outproj results are DMAed straight from PSUM as f32.
            with (
                tc.tile_pool(name="op", bufs=3, space="PSUM") as op_,
                tc.tile_pool(name="vtp", bufs=1, space="PSUM") as vtpp,
                tc.tile_pool(name="opps", bufs=3, space="PSUM") as opps,
                tc.tile_pool(name="ptp", bufs=1, space="PSUM") as ptp,
                tc.tile_pool(name="snapp", bufs=4) as snapp,
                tc.tile_pool(name="recp", bufs=8) as recp,
                tc.tile_pool(name="vhrp", bufs=5) as vhrp,
                tc.tile_pool(name="obp", bufs=3) as obp,
            ):

                def scores_mf(sp):
                    qs = slice(sp * SPAN, (sp + 1) * SPAN)
                    cka = slice(sp * SPAN, sp * SPAN + CH)
                    ckb = slice(sp * SPAN + CH, (sp + 1) * SPAN)
                    for h in range(HPC):
                        hs = slice(h * 64, (h + 1) * 64)
                        ptj = ptp.tile([128, 384], F32, tag="ptj", name="ptj")
                        nc.tensor.matmul(
                            ptj[:, 0:SPAN], kh2[hs, cka], qh2[hs, qs],
                            start=True, stop=True, tile_position=(h * 64, 0),
                        )
                        nc.tensor.matmul(
                            ptj[:, SPAN:384], kh2[hs, ckb], qh2[hs, ckb],
                            start=True, stop=True, tile_position=(h * 64, 0),
                        )
                        # mf = (ptj * s) * [tri|ones|tri]; tri pair on DVE,
                        # unmasked offdiag block on Act.
                        mf = sctp.tile(
                            [128, 384], BF, tag=f"mf{h}_{sp}", name=f"mf{h}_{sp}"
                        )
                        tri2_in = bass.AP(
                            ptj[:].tensor, ptj[:].offset, [[384, 128], [256, 2], [1, 128]]
                        )
                        tri2_msk = bass.AP(
                            consts[:].tensor,
                            consts[:].offset + 256,
                            [[644, 128], [256, 2], [1, 128]],
                        )
                        tri2_out = bass.AP(
                            mf[:].tensor, mf[:].offset, [[384, 128], [256, 2], [1, 128]]
                        )
                        nc.vector.scalar_tensor_tensor(
                            tri2_out, tri2_in, scv[h][:], tri2_msk,
                            AluOpType.mult, AluOpType.mult,
                        )
                        acct("v", 256)
                        if pick(128) == "v":
                            nc.vector.tensor_scalar_mul(
                                mf[:, 128:256], ptj[:, 128:256], scv[h][:]
                            )
                        else:
                            nc.scalar.activation(
                                mf[:, 128:256], ptj[:, 128:256], AF.Copy, scale=scv[h][:]
                            )
                        mfs[(sp, h)] = mf

                def snap_prep(sp):
                    # scale state snapshot k-dim rows 0:64 by s (row 64 unscaled)
                    for h in range(HPC):
                        su = s_chain[(h, sp - 1)]
                        snap = snapp.tile([128, 65], BF, tag=f"snap{h}", name=f"snap{h}")
                        sview = snap[h * 64 : (h + 1) * 64, :]
                        if pick(65, False) == "v":
                            nc.vector.tensor_scalar_mul(
                                sview, su[0:64, :], scv[h][0:64, :]
                            )
                        else:
                            nc.scalar.activation(
                                sview, su[0:64, :], AF.Copy, scale=scv[h][0:64, :]
                            )
                        snapo = snapp.tile([1, 65], BF, tag=f"snapo{h}", name=f"snapo{h}")
                        nc.scalar.copy(snapo[:], su[64:65, :])
                        acct("s", 65, False)
                        snaps[(sp, h)] = (snap, snapo)

                def vht_outproj_chunk(cidx, vhr, split_dma=False):
                    ck = slice(cidx * CH, (cidx + 1) * CH)
                    vtp = vtpp.tile([128, CH], BF, tag="vtp", name="vtp")
                    nc.tensor.transpose(vtp[:], vhr[:], ident)
                    rot(vht[:, ck], vtp[:], 128, psum_src=True)
                    ob = obp.tile([128, D_MODEL], BF, tag="ob", name="ob")
                    for n2 in range(D_MODEL // 512):
                        ns = slice(n2 * 512, (n2 + 1) * 512)
                        opp = opps.tile([128, 512], F32, tag="opp", name="opp")
                        nc.tensor.matmul(
                            opp[:], vht[:, ck], wo_sb[:, ns], start=True, stop=True
                        )
                        rot(ob[:, ns], opp[:], 512)
                        if split_dma:
                            nc.sync.dma_start(out_d[ck, ns], ob[:, ns])
                    if not split_dma:
                        nc.sync.dma_start(out_d[ck, :], ob[:])

                def vht_outproj(sp, vhrs):
                    for cidx in (2 * sp, 2 * sp + 1):
                        vht_outproj_chunk(cidx, vhrs[cidx])

                prev_vhrs = None
                snaps = {}
                scores_mf(0)
                scores_mf(1)
                snap_prep(1)
                for sp in range(NSPAN):
                    qs = slice(sp * SPAN, (sp + 1) * SPAN)
                    cka = slice(sp * SPAN, sp * SPAN + CH)
                    ckb = slice(sp * SPAN + CH, (sp + 1) * SPAN)
                    ca, cb = 2 * sp, 2 * sp + 1
                    vhrs = {
                        ca: vhrp.tile([128, 128], BF, tag="vhra", name="vhra"),
                        cb: vhrp.tile([128, 128], BF, tag="vhrb", name="vhrb"),
                    }
                    def attn_chunk(cidx, h):
                        vra = vrows[:, ca * RST + h * 65 : ca * RST + h * 65 + 65]
                        vrb = vrows[:, cb * RST + h * 65 : cb * RST + h * 65 + 65]
                        mf = mfs[(sp, h)]
                        if cidx == ca:
                            ck, parts = cka, ((tri, vra), (mf[:, 0:CH], vra))
                        else:
                            ck, parts = ckb, (
                                (ones128, vra),
                                (tri, vrb),
                                (mf[:, CH : 2 * CH], vra),
                                (mf[:, 2 * CH : 3 * CH], vrb),
                            )
                        o = op_.tile([128, 65], F32, tag="o", name="o")
                        nmm = len(parts) + (2 if sp > 0 else 0)
                        for mi, (mm, vv) in enumerate(parts):
                            nc.tensor.matmul(
                                o[:], mm, vv, start=(mi == 0), stop=(mi == nmm - 1)
                            )
                        if sp > 0:
                            snap, snapo = snaps[(sp, h)]
                            nc.tensor.matmul(
                                o[:],
                                qh2[h * 64 : (h + 1) * 64, ck],
                                snap[h * 64 : (h + 1) * 64, :],
                                start=False,
                                stop=False,
                                tile_position=(h * 64, 0),
                            )
                            nc.tensor.matmul(
                                o[:], onesall[0:1, 0:128], snapo[:],
                                start=False, stop=True,
                            )
                        rec = recp.tile([128, 1], F32, tag="rec", name="rec")
                        nc.vector.reciprocal(rec[:], o[:, 64:65])
                        acct("v", 1)
                        dst = vhrs[cidx][:, h * 64 : (h + 1) * 64]
                        if pick(64) == "v":
                            rec_b = bass.AP(
                                rec[:].tensor, rec[:].offset, [[1, 128], [0, 64]]
                            )
                            nc.vector.tensor_mul(dst, o[:, 0:64], rec_b)
                        else:
                            nc.scalar.activation(dst, o[:, 0:64], AF.Copy, scale=rec[:])

                    if sp < NSPAN - 1:
                        for h in range(HPC):
                            attn_chunk(ca, h)
                            attn_chunk(cb, h)
                        if sp + 2 < NSPAN:
                            scores_mf(sp + 2)
                        if sp + 1 < NSPAN:
                            snap_prep(sp + 1)
                        if prev_vhrs is not None:
                            vht_outproj(sp - 1, prev_vhrs)
                        prev_vhrs = vhrs
                    else:
                        # last span: chunk-major, outproj immediately per chunk
                        vht_outproj(sp - 1, prev_vhrs)
                        for cidx in (ca, cb):
                            for h in range(HPC):
                                attn_chunk(cidx, h)
                            vht_outproj_chunk(cidx, vhrs[cidx], split_dma=(cidx == cb))

    nc.compile()
    return nc


def _consts():
    import ml_dtypes

    bf = ml_dtypes.bfloat16
    consts = np.zeros((128, 644), dtype=np.float32)
    consts[:, 0:128] = np.eye(128)
    consts[:, 128:256] = 1.0
    j = np.arange(128)[:, None]
    i = np.arange(128)[None, :]
    tri = (j <= i).astype(np.float32)
    consts[:, 256:384] = tri
    consts[:, 384:512] = 1.0
    consts[:, 512:640] = tri
    for h in range(HPC):
        consts[h * 64 : (h + 1) * 64, 640 + h] = 1.0
    return consts.astype(bf)


def _in_maps(inputs):
    import ml_dtypes

    bf = ml_dtypes.bfloat16
    X = np.ascontiguousarray(np.asarray(inputs["X"], dtype=np.float32))
    xt = np.ascontiguousarray(X[0].T).astype(bf)  # [D_MODEL, N]
    wqt = np.ascontiguousarray(np.asarray(inputs["Wq"], np.float32).T).astype(bf)
    wkt = np.ascontiguousarray(np.asarray(inputs["Wk"], np.float32).T).astype(bf)
    wvt = np.ascontiguousarray(np.asarray(inputs["Wv"], np.float32).T).astype(bf)
    wot = np.ascontiguousarray(np.asarray(inputs["Wo"], np.float32).T).astype(bf)
    consts = _consts()

    def sb_layout(w):  # [1024, 128] -> [128, 8*128] (dm-chunk on partitions)
        return np.ascontiguousarray(
            w.reshape(KT, 128, DPC).transpose(1, 0, 2).reshape(128, KT * DPC)
        )

    in_maps = []
    for c in range(NCORES):
        cs = slice(c * DPC, (c + 1) * DPC)
        in_maps.append(
            {
                "xt": xt,
                "wq": sb_layout(wqt[:, cs]),
                "wk": sb_layout(wkt[:, cs]),
                "wv": sb_layout(wvt[:, cs]),
                "wo": np.ascontiguousarray(wot[cs, :]),
                "consts": consts,
            }
        )
    return in_maps


def _run(inputs, trace=False):
    from concourse.bass_utils import run_bass_kernel_spmd

    if "nc" not in _CACHE:
        _CACHE["nc"] = _build()
    nc = _CACHE["nc"]
    in_maps = _in_maps(inputs)
    res = run_bass_kernel_spmd(nc, in_maps, core_ids=list(range(NCORES)), trace=trace)
    bo = np.asarray(inputs["bo"], dtype=np.float32)
    acc = np.zeros((N, D_MODEL), dtype=np.float32)
    for c in range(NCORES):
        acc += res.results[c]["out"].astype(np.float32)
    acc += bo[None, :]
    return acc.reshape(B, N, D_MODEL), res.exec_time_ns


def kernel(**inputs) -> np.ndarray:
    out, _ = _run(inputs, trace=False)
    return out
